# revision 9
# baseline (speedup 1.0000x reference)
"""Mixtral MoE layer (8 experts, top-2, H=2048, I=7168, T=8192) on 8 trn2 NeuronCores.

Intermediate-sharded SPMD: every core processes ALL experts' routed tokens but
only a 1/8 slice of the intermediate dim (896 of 7168). Phase A computes its
G-slice = silu(x@w1s.T) * (x@w3s.T); phase B contracts the slice against w2s
into a PARTIAL output accumulated fully in PSUM (7-matmul groups). The host
sums the 8 partial outputs and applies routing weights.

Why: per-core work is identical by construction (no max-expert padding — the
per-expert token counts are baked into the program at build time), phase B
needs no vector-engine accumulation, and weights stream per 512-token block so
everything double-buffers. bf16 matmuls, fp32 PSUM, bf16 partial outputs.
"""

import math

import numpy as np
import ml_dtypes

import concourse.bass as bass
import concourse.mybir as mybir
import concourse.tile as tile
from concourse.bass_utils import run_bass_kernel_spmd

H = 2048          # hidden dim
I = 7168          # intermediate dim
E = 8             # experts
NCORES = 8
IS = I // NCORES  # 896 per-core intermediate slice
ISC = IS // 128   # 7 chunks of 128
HJ = H // 128     # 16 hidden chunks of 128
TOPK = 2
TBMAX = 512       # token block (matmul free dim, one PSUM bank)
# i-groups within the 896 slice for w1/w3 streaming (chunks of 256 + tail 128)
IGS = [(0, 256), (256, 256), (512, 256), (768, 128)]

BF16 = mybir.dt.bfloat16
F32 = mybir.dt.float32

last_exec_time_ns = None  # set when BASS_MOE_TRACE=1
last_results = None


def _install_axon_hooks_shim():
    """This image lacks antenv.axon_hooks (needed by run_bass_kernel_spmd
    trace=True). Provide it, with the NTFF profile hook driven via ctypes
    into the injected axon .so (mirrors trn_agent_boot._ntff_profile_via_ctypes)."""
    import sys

    try:
        import antenv.axon_hooks  # noqa: F401

        return
    except ImportError:
        pass
    import contextlib
    import ctypes
    import types

    hook = None
    so_path = "/opt/axon/libaxon_pjrt.so"
    try:
        lib = ctypes.CDLL(so_path)
        if hasattr(lib, "axon_start_nrt_profile"):
            lib.axon_start_nrt_profile.argtypes = [
                ctypes.POINTER(ctypes.c_int64),
                ctypes.c_size_t,
            ]
            lib.axon_start_nrt_profile.restype = ctypes.c_int64
            lib.axon_stop_nrt_profile.argtypes = [ctypes.c_char_p]
            lib.axon_stop_nrt_profile.restype = ctypes.c_int64

            @contextlib.contextmanager
            def _hook(output_dir, device_ids):
                import jax

                jax.devices()
                if device_ids:
                    ids = (ctypes.c_int64 * len(device_ids))(*device_ids)
                    rc = lib.axon_start_nrt_profile(ids, len(device_ids))
                else:
                    rc = lib.axon_start_nrt_profile(None, 0)
                if rc != 0:
                    raise RuntimeError(f"axon_start_nrt_profile rc={rc}")
                try:
                    yield
                finally:
                    n = lib.axon_stop_nrt_profile(str(output_dir).encode())
                    print(f"ntff profile: {n} file(s) -> {output_dir}", flush=True)

            hook = _hook
    except OSError:
        pass

    mod = types.ModuleType("antenv.axon_hooks")
    mod._hook = hook
    mod.get_axon_ntff_profile_hook = lambda: mod._hook
    mod.set_axon_ntff_profile_hook = lambda h: setattr(mod, "_hook", h)
    sys.modules["antenv.axon_hooks"] = mod


_install_axon_hooks_shim()


def legalize_single_wait(nc):
    """This walrus rejects >1 sem wait per instruction: hoist extras onto
    preceding NoOps on the same engine (per-engine program order preserved)."""
    n_split = 0
    for fn in nc.m.functions:
        for blk in fn.blocks:
            new = []
            for inst in blk.instructions:
                si = inst.sync_info
                if si is not None and si.on_wait and len(si.on_wait) > 1:
                    waits = list(si.on_wait)
                    for i, w in enumerate(waits[:-1]):
                        nop = mybir.InstNoOp(name=f"{inst.name}-w{i}", ins=[], outs=[])
                        nop.engine = inst.engine
                        nop.sync_info = mybir.SyncInfo(on_wait=[w], on_update=[])
                        new.append(nop)
                        n_split += 1
                    inst.sync_info = mybir.SyncInfo(
                        on_wait=[waits[-1]], on_update=list(si.on_update)
                    )
                new.append(inst)
            blk.instructions[:] = new
    return n_split


def _block_sizes(n):
    """Near-equal 16-aligned blocks <= TBMAX covering n (n a multiple of 16)."""
    if n == 0:
        return []
    nb = math.ceil(n / TBMAX)
    base = (n // nb) // 16 * 16
    rem = (n - base * nb) // 16
    return [base + (16 if i < rem else 0) for i in range(nb)]


def _super_blocks(counts):
    """Per expert: column groups <=512 paired into super-blocks sharing one
    weight-streaming pass. Expert 0 leads with a small 128-token group so the
    first matmul only waits on ~1.5MB of DMA."""
    sbs = []  # (expert, global offset, [group widths])
    off = 0
    for e in range(E):
        n = counts[e]
        groups = []
        if e == 0 and n > 256:
            groups.append(128)
            groups.extend(_block_sizes(n - 128))
        else:
            groups.extend(_block_sizes(n))
        i = 0
        if e == 0 and len(groups) % 2 == 1:
            sbs.append((e, off, [groups[0]]))
            off += groups[0]
            i = 1
        while i < len(groups):
            pair = groups[i : i + 2]
            sbs.append((e, off, pair))
            off += sum(pair)
            i += 2
    return sbs


_programs = {}


def _build_program(counts):
    """One SPMD program: all experts' FFN on a 1/8 inter-slice.

    counts: tuple of 8 padded (mult-of-16) per-expert token counts."""
    key = tuple(counts)
    if key in _programs:
        return _programs[key]

    TT = sum(counts)
    nc = bass.Bass("TRN2", target_bir_lowering=False, debug=False, num_devices=NCORES)
    xt = nc.declare_dram_parameter("xt", [HJ, 128, TT], BF16, isOutput=False)
    w1 = nc.declare_dram_parameter("w1", [E, HJ, 128, IS], BF16, isOutput=False)
    w3 = nc.declare_dram_parameter("w3", [E, HJ, 128, IS], BF16, isOutput=False)
    w2 = nc.declare_dram_parameter("w2", [E, ISC, 128, H], BF16, isOutput=False)
    yt = nc.declare_dram_parameter("yt", [HJ, 128, TT], BF16, isOutput=True)

    # flat per-expert blocks, each its own weight-streaming pass; expert 0
    # leads with a small block so the first matmul waits on ~1.5MB of DMA
    sbs = []
    off = 0
    for e in range(E):
        n = counts[e]
        groups = [128] + _block_sizes(n - 128) if e == 0 and n > 256 else _block_sizes(n)
        for tb in groups:
            sbs.append((e, off, [tb]))
            off += tb
    assert off == TT

    with tile.TileContext(nc) as tc:
        with (
            tc.tile_pool(name="xp", bufs=3) as xp,
            tc.tile_pool(name="w1p", bufs=4) as w1p,
            tc.tile_pool(name="w3p", bufs=4) as w3p,
            tc.tile_pool(name="w2p", bufs=3) as w2p,
            tc.tile_pool(name="gtp", bufs=2 * ISC) as gtp,
            tc.tile_pool(name="sip", bufs=3) as sip,
            tc.tile_pool(name="otp", bufs=6) as otp,
            tc.tile_pool(name="pga", bufs=2, space="PSUM") as pga,
            tc.tile_pool(name="pob", bufs=4, space="PSUM") as pob,
        ):
            for e, c0, groups in sbs:
                sbw = sum(groups)
                goffs = [c0 + sum(groups[:i]) for i in range(len(groups))]
                xsb = xp.tile([128, HJ, sbw], BF16, tag="xsb")
                nc.sync.dma_start(
                    out=xsb[:, :, :],
                    in_=xt[:, :, c0 : c0 + sbw].rearrange("j p c -> p j c"),
                )

                # ---- phase A: G slice [896, sbw]; one weight pass feeds
                # every column group ----
                gts = [[] for _ in groups]
                for i0, gw in IGS:
                    w1sb = w1p.tile([128, HJ, 256], BF16, tag="w1sb")
                    nc.sync.dma_start(
                        out=w1sb[:, :, :gw],
                        in_=w1[e, :, :, i0 : i0 + gw].rearrange("j p i -> p j i"),
                    )
                    w3sb = w3p.tile([128, HJ, 256], BF16, tag="w3sb")
                    nc.scalar.dma_start(
                        out=w3sb[:, :, :gw],
                        in_=w3[e, :, :, i0 : i0 + gw].rearrange("j p i -> p j i"),
                    )
                    for m in range(gw // 128):
                        ms = slice(m * 128, (m + 1) * 128)
                        for g, tb in enumerate(groups):
                            cs = slice(goffs[g] - c0, goffs[g] - c0 + tb)
                            pg1 = pga.tile([128, tb], F32, tag="pg1")
                            pg3 = pga.tile([128, tb], F32, tag="pg3")
                            for k in range(HJ):
                                nc.tensor.matmul(
                                    pg1[:, :],
                                    lhsT=w1sb[:, k, ms],
                                    rhs=xsb[:, k, cs],
                                    start=(k == 0),
                                    stop=(k == HJ - 1),
                                )
                            for k in range(HJ):
                                nc.tensor.matmul(
                                    pg3[:, :],
                                    lhsT=w3sb[:, k, ms],
                                    rhs=xsb[:, k, cs],
                                    start=(k == 0),
                                    stop=(k == HJ - 1),
                                )
                            ssb = sip.tile([128, tb], F32, tag="ssb")
                            nc.scalar.activation(
                                ssb[:, :], pg1[:, :], mybir.ActivationFunctionType.Silu
                            )
                            gt = gtp.tile([128, tb], BF16, tag="gt")
                            nc.vector.tensor_mul(gt[:, :], pg3[:, :], ssb[:, :])
                            gts[g].append(gt)

                # ---- phase B: partial Y[2048, sbw], contraction over the 896
                # slice entirely in PSUM (7-matmul groups) ----
                for hg in range(4):
                    w2sb = w2p.tile([128, ISC, 512], BF16, tag="w2sb")
                    nc.gpsimd.dma_start(
                        out=w2sb[:, :, :],
                        in_=w2[e, :, :, hg * 512 : (hg + 1) * 512].rearrange(
                            "c p h -> p c h"
                        ),
                    )
                    for g, tb in enumerate(groups):
                        for hh in range(4):
                            po = pob.tile([128, tb], F32, tag="po")
                            hs = slice(hh * 128, (hh + 1) * 128)
                            for u in range(ISC):
                                nc.tensor.matmul(
                                    po[:, :],
                                    lhsT=w2sb[:, u, hs],
                                    rhs=gts[g][u][:, :],
                                    start=(u == 0),
                                    stop=(u == ISC - 1),
                                )
                            ot = otp.tile([128, tb], BF16, tag="ot")
                            nc.vector.tensor_copy(ot[:, :], po[:, :])
                            nc.gpsimd.dma_start(
                                out=yt[hg * 4 + hh, :, goffs[g] : goffs[g] + tb],
                                in_=ot[:, :],
                            )

    legalize_single_wait(nc)
    _programs[key] = nc
    return nc


def _routing(x, gate_weight):
    """Replicate the reference router bitwise-closely: jax on CPU, same ops."""
    import jax
    import jax.numpy as jnp

    cpu = jax.devices("cpu")[0]
    with jax.default_device(cpu):
        router_logits = jnp.asarray(x) @ jnp.asarray(gate_weight).T
        probs = jax.nn.softmax(router_logits.astype(jnp.float32), axis=-1)
        top_w, top_idx = jax.lax.top_k(probs, TOPK)
        top_w = top_w / jnp.sum(top_w, axis=-1, keepdims=True)
        top_w = top_w.astype(x.dtype)
        return np.asarray(top_w), np.asarray(top_idx)


def kernel(hidden_states, gate_weight, w1_weight, w3_weight, w2_weight):
    import os

    bf16 = ml_dtypes.bfloat16
    x = np.asarray(hidden_states, dtype=np.float32)
    T = x.shape[0]
    top_w, top_idx = _routing(x, np.asarray(gate_weight, dtype=np.float32))

    tok_ids = []
    tok_w = []
    counts = []
    for e in range(E):
        rows, cols = np.nonzero(top_idx == e)
        tok_ids.append(rows)
        tok_w.append(top_w[rows, cols].astype(np.float32))
        counts.append(max(16, math.ceil(len(rows) / 16) * 16))
    TT = sum(counts)
    offs = np.cumsum([0] + counts)

    # gathered, expert-concatenated tokens, transposed for contiguous DMA runs
    xcat = np.zeros((TT, H), dtype=bf16)
    for e in range(E):
        xcat[offs[e] : offs[e] + len(tok_ids[e])] = x[tok_ids[e]]
    xt = np.ascontiguousarray(xcat.T).reshape(HJ, 128, TT)

    # per-core I-slices of all experts' weights, pre-transposed
    # w1/w3: [E, I, H] -> per core [E, HJ, 128, IS]
    w1b = np.asarray(w1_weight, dtype=bf16).reshape(E, NCORES, IS, HJ, 128)
    w1c = np.ascontiguousarray(w1b.transpose(1, 0, 3, 4, 2))  # [core,E,HJ,128,IS]
    w3b = np.asarray(w3_weight, dtype=bf16).reshape(E, NCORES, IS, HJ, 128)
    w3c = np.ascontiguousarray(w3b.transpose(1, 0, 3, 4, 2))
    # w2: [E, H, I] -> per core [E, ISC, 128, H]
    w2b = np.asarray(w2_weight, dtype=bf16).reshape(E, H, NCORES, ISC, 128)
    w2c = np.ascontiguousarray(w2b.transpose(2, 0, 3, 4, 1))  # [core,E,ISC,128,H]

    in_maps = [
        {"xt": xt, "w1": w1c[j], "w3": w3c[j], "w2": w2c[j]} for j in range(NCORES)
    ]

    nc = _build_program(tuple(counts))
    trace = os.environ.get("BASS_MOE_TRACE", "") == "1"
    res = None
    if trace:
        import concourse.bass_utils as bu

        orig_upload = bu.upload_artifacts
        bu.upload_artifacts = lambda tmpdir: f"local://{tmpdir}"
        tdir = os.environ.get("BASS_MOE_TRACE_DIR") or None
        try:
            res = run_bass_kernel_spmd(
                nc, in_maps, list(range(NCORES)), trace=True, tmpdir=tdir
            )
        except Exception as exc:
            print(f"trace path failed ({type(exc).__name__}: {exc}); rerunning untraced", flush=True)
            res = None
        finally:
            bu.upload_artifacts = orig_upload
    if res is None:
        res = run_bass_kernel_spmd(nc, in_maps, list(range(NCORES)))
    global last_exec_time_ns, last_results
    last_exec_time_ns = res.exec_time_ns
    last_results = res

    # host combine: sum the 8 partial outputs, then weighted scatter-add
    ysum = np.zeros((H, TT), dtype=np.float32)
    for j in range(NCORES):
        ysum += res.results[j]["yt"].reshape(H, TT).astype(np.float32)

    out = np.zeros((T, H), dtype=np.float32)
    for e in range(E):
        n_e = len(tok_ids[e])
        seg = ysum[:, offs[e] : offs[e] + n_e]
        out[tok_ids[e]] += tok_w[e][:, None] * seg.T
    return out


# revision 10
# speedup vs baseline: 1.1489x; 1.1489x over previous
"""Mixtral MoE layer (8 experts, top-2, H=2048, I=7168, T=8192) on 8 trn2 NeuronCores.

Intermediate-sharded SPMD: every core processes ALL experts' routed tokens but
only a 1/8 slice of the intermediate dim (896 of 7168). Phase A computes its
G-slice = silu(x@w1s.T) * (x@w3s.T); phase B contracts the slice against w2s
into a PARTIAL output accumulated fully in PSUM (7-matmul groups). The host
sums the 8 partial outputs and applies routing weights.

Why: per-core work is identical by construction (no max-expert padding — the
per-expert token counts are baked into the program at build time), phase B
needs no vector-engine accumulation, and weights stream per 512-token block so
everything double-buffers. bf16 matmuls, fp32 PSUM, bf16 partial outputs.
"""

import math

import numpy as np
import ml_dtypes

import concourse.bass as bass
import concourse.mybir as mybir
import concourse.tile as tile
from concourse.bass_utils import run_bass_kernel_spmd

H = 2048          # hidden dim
I = 7168          # intermediate dim
E = 8             # experts
NCORES = 8
IS = I // NCORES  # 896 per-core intermediate slice
ISC = IS // 128   # 7 chunks of 128
HJ = H // 128     # 16 hidden chunks of 128
TOPK = 2
TBMAX = 512       # token block (matmul free dim, one PSUM bank)
# i-groups within the 896 slice for w1/w3 streaming (chunks of 256 + tail 128)
IGS = [(0, 256), (256, 256), (512, 256), (768, 128)]

BF16 = mybir.dt.bfloat16
F32 = mybir.dt.float32

last_exec_time_ns = None  # set when BASS_MOE_TRACE=1
last_results = None


def _install_axon_hooks_shim():
    """This image lacks antenv.axon_hooks (needed by run_bass_kernel_spmd
    trace=True). Provide it, with the NTFF profile hook driven via ctypes
    into the injected axon .so (mirrors trn_agent_boot._ntff_profile_via_ctypes)."""
    import sys

    try:
        import antenv.axon_hooks  # noqa: F401

        return
    except ImportError:
        pass
    import contextlib
    import ctypes
    import types

    hook = None
    so_path = "/opt/axon/libaxon_pjrt.so"
    try:
        lib = ctypes.CDLL(so_path)
        if hasattr(lib, "axon_start_nrt_profile"):
            lib.axon_start_nrt_profile.argtypes = [
                ctypes.POINTER(ctypes.c_int64),
                ctypes.c_size_t,
            ]
            lib.axon_start_nrt_profile.restype = ctypes.c_int64
            lib.axon_stop_nrt_profile.argtypes = [ctypes.c_char_p]
            lib.axon_stop_nrt_profile.restype = ctypes.c_int64

            @contextlib.contextmanager
            def _hook(output_dir, device_ids):
                import jax

                jax.devices()
                if device_ids:
                    ids = (ctypes.c_int64 * len(device_ids))(*device_ids)
                    rc = lib.axon_start_nrt_profile(ids, len(device_ids))
                else:
                    rc = lib.axon_start_nrt_profile(None, 0)
                if rc != 0:
                    raise RuntimeError(f"axon_start_nrt_profile rc={rc}")
                try:
                    yield
                finally:
                    n = lib.axon_stop_nrt_profile(str(output_dir).encode())
                    print(f"ntff profile: {n} file(s) -> {output_dir}", flush=True)

            hook = _hook
    except OSError:
        pass

    mod = types.ModuleType("antenv.axon_hooks")
    mod._hook = hook
    mod.get_axon_ntff_profile_hook = lambda: mod._hook
    mod.set_axon_ntff_profile_hook = lambda h: setattr(mod, "_hook", h)
    sys.modules["antenv.axon_hooks"] = mod


_install_axon_hooks_shim()


def legalize_single_wait(nc):
    """This walrus rejects >1 sem wait per instruction: hoist extras onto
    preceding NoOps on the same engine (per-engine program order preserved)."""
    n_split = 0
    for fn in nc.m.functions:
        for blk in fn.blocks:
            new = []
            for inst in blk.instructions:
                si = inst.sync_info
                if si is not None and si.on_wait and len(si.on_wait) > 1:
                    waits = list(si.on_wait)
                    for i, w in enumerate(waits[:-1]):
                        nop = mybir.InstNoOp(name=f"{inst.name}-w{i}", ins=[], outs=[])
                        nop.engine = inst.engine
                        nop.sync_info = mybir.SyncInfo(on_wait=[w], on_update=[])
                        new.append(nop)
                        n_split += 1
                    inst.sync_info = mybir.SyncInfo(
                        on_wait=[waits[-1]], on_update=list(si.on_update)
                    )
                new.append(inst)
            blk.instructions[:] = new
    return n_split


def _block_sizes(n):
    """Near-equal 16-aligned blocks <= TBMAX covering n (n a multiple of 16)."""
    if n == 0:
        return []
    nb = math.ceil(n / TBMAX)
    base = (n // nb) // 16 * 16
    rem = (n - base * nb) // 16
    return [base + (16 if i < rem else 0) for i in range(nb)]


_programs = {}


def _build_program(counts):
    """One SPMD program: all experts' FFN on a 1/8 inter-slice.

    counts: tuple of 8 padded (mult-of-16) per-expert token counts."""
    key = tuple(counts)
    if key in _programs:
        return _programs[key]

    TT = sum(counts)
    nc = bass.Bass("TRN2", target_bir_lowering=False, debug=False, num_devices=NCORES)
    xt = nc.declare_dram_parameter("xt", [HJ, 128, TT], BF16, isOutput=False)
    w1 = nc.declare_dram_parameter("w1", [E, HJ, 128, IS], BF16, isOutput=False)
    w3 = nc.declare_dram_parameter("w3", [E, HJ, 128, IS], BF16, isOutput=False)
    w2 = nc.declare_dram_parameter("w2", [E, ISC, 128, H], BF16, isOutput=False)
    yt = nc.declare_dram_parameter("yt", [HJ, 128, TT], BF16, isOutput=True)

    # flat per-expert blocks, each its own weight-streaming pass; expert 0
    # leads with a small block so the first matmul waits on ~1.5MB of DMA
    sbs = []
    off = 0
    for e in range(E):
        n = counts[e]
        groups = [128] + _block_sizes(n - 128) if e == 0 and n > 256 else _block_sizes(n)
        for tb in groups:
            sbs.append((e, off, [tb]))
            off += tb
    assert off == TT

    with tile.TileContext(nc) as tc:
        with (
            tc.tile_pool(name="xp", bufs=3) as xp,
            tc.tile_pool(name="w1p", bufs=4) as w1p,
            tc.tile_pool(name="w3p", bufs=4) as w3p,
            tc.tile_pool(name="w2p", bufs=3) as w2p,
            tc.tile_pool(name="gtp", bufs=2 * ISC) as gtp,
            tc.tile_pool(name="sip", bufs=3) as sip,
            tc.tile_pool(name="otp", bufs=6) as otp,
            tc.tile_pool(name="pga", bufs=2, space="PSUM") as pga,
            tc.tile_pool(name="pob", bufs=4, space="PSUM") as pob,
        ):
            for e, c0, groups in sbs:
                sbw = sum(groups)
                goffs = [c0 + sum(groups[:i]) for i in range(len(groups))]
                xsb = xp.tile([128, HJ, sbw], BF16, tag="xsb")
                nc.sync.dma_start(
                    out=xsb[:, :, :],
                    in_=xt[:, :, c0 : c0 + sbw].rearrange("j p c -> p j c"),
                )

                # ---- phase A: G slice [896, sbw]; one weight pass feeds
                # every column group ----
                gts = [[] for _ in groups]
                for i0, gw in IGS:
                    w1sb = w1p.tile([128, HJ, 256], BF16, tag="w1sb")
                    nc.sync.dma_start(
                        out=w1sb[:, :, :gw],
                        in_=w1[e, :, :, i0 : i0 + gw].rearrange("j p i -> p j i"),
                    )
                    w3sb = w3p.tile([128, HJ, 256], BF16, tag="w3sb")
                    nc.scalar.dma_start(
                        out=w3sb[:, :, :gw],
                        in_=w3[e, :, :, i0 : i0 + gw].rearrange("j p i -> p j i"),
                    )
                    for m in range(gw // 128):
                        ms = slice(m * 128, (m + 1) * 128)
                        for g, tb in enumerate(groups):
                            cs = slice(goffs[g] - c0, goffs[g] - c0 + tb)
                            pg1 = pga.tile([128, tb], F32, tag="pg1")
                            pg3 = pga.tile([128, tb], F32, tag="pg3")
                            for k in range(HJ):
                                nc.tensor.matmul(
                                    pg1[:, :],
                                    lhsT=w1sb[:, k, ms],
                                    rhs=xsb[:, k, cs],
                                    start=(k == 0),
                                    stop=(k == HJ - 1),
                                )
                            for k in range(HJ):
                                nc.tensor.matmul(
                                    pg3[:, :],
                                    lhsT=w3sb[:, k, ms],
                                    rhs=xsb[:, k, cs],
                                    start=(k == 0),
                                    stop=(k == HJ - 1),
                                )
                            ssb = sip.tile([128, tb], F32, tag="ssb")
                            nc.scalar.activation(
                                ssb[:, :], pg1[:, :], mybir.ActivationFunctionType.Silu
                            )
                            gt = gtp.tile([128, tb], BF16, tag="gt")
                            nc.vector.tensor_mul(gt[:, :], pg3[:, :], ssb[:, :])
                            gts[g].append(gt)

                # ---- phase B: partial Y[2048, sbw], contraction over the 896
                # slice entirely in PSUM (7-matmul groups) ----
                for hg in range(4):
                    w2sb = w2p.tile([128, ISC, 512], BF16, tag="w2sb")
                    nc.gpsimd.dma_start(
                        out=w2sb[:, :, :],
                        in_=w2[e, :, :, hg * 512 : (hg + 1) * 512].rearrange(
                            "c p h -> p c h"
                        ),
                    )
                    for g, tb in enumerate(groups):
                        for hh in range(4):
                            po = pob.tile([128, tb], F32, tag="po")
                            hs = slice(hh * 128, (hh + 1) * 128)
                            for u in range(ISC):
                                nc.tensor.matmul(
                                    po[:, :],
                                    lhsT=w2sb[:, u, hs],
                                    rhs=gts[g][u][:, :],
                                    start=(u == 0),
                                    stop=(u == ISC - 1),
                                )
                            ot = otp.tile([128, tb], BF16, tag="ot")
                            nc.vector.tensor_copy(ot[:, :], po[:, :])
                            nc.gpsimd.dma_start(
                                out=yt[hg * 4 + hh, :, goffs[g] : goffs[g] + tb],
                                in_=ot[:, :],
                            )

    legalize_single_wait(nc)
    _programs[key] = nc
    return nc


def _routing(x, gate_weight):
    """Replicate the reference router bitwise-closely: jax on CPU, same ops."""
    import jax
    import jax.numpy as jnp

    cpu = jax.devices("cpu")[0]
    with jax.default_device(cpu):
        router_logits = jnp.asarray(x) @ jnp.asarray(gate_weight).T
        probs = jax.nn.softmax(router_logits.astype(jnp.float32), axis=-1)
        top_w, top_idx = jax.lax.top_k(probs, TOPK)
        top_w = top_w / jnp.sum(top_w, axis=-1, keepdims=True)
        top_w = top_w.astype(x.dtype)
        return np.asarray(top_w), np.asarray(top_idx)


def kernel(hidden_states, gate_weight, w1_weight, w3_weight, w2_weight):
    import os

    bf16 = ml_dtypes.bfloat16
    x = np.asarray(hidden_states, dtype=np.float32)
    T = x.shape[0]
    top_w, top_idx = _routing(x, np.asarray(gate_weight, dtype=np.float32))

    tok_ids = []
    tok_w = []
    counts = []
    for e in range(E):
        rows, cols = np.nonzero(top_idx == e)
        tok_ids.append(rows)
        tok_w.append(top_w[rows, cols].astype(np.float32))
        counts.append(max(16, math.ceil(len(rows) / 16) * 16))
    TT = sum(counts)
    offs = np.cumsum([0] + counts)

    # gathered, expert-concatenated tokens, transposed for contiguous DMA runs
    xcat = np.zeros((TT, H), dtype=bf16)
    for e in range(E):
        xcat[offs[e] : offs[e] + len(tok_ids[e])] = x[tok_ids[e]]
    xt = np.ascontiguousarray(xcat.T).reshape(HJ, 128, TT)

    # per-core I-slices of all experts' weights, pre-transposed
    # w1/w3: [E, I, H] -> per core [E, HJ, 128, IS]
    w1b = np.asarray(w1_weight, dtype=bf16).reshape(E, NCORES, IS, HJ, 128)
    w1c = np.ascontiguousarray(w1b.transpose(1, 0, 3, 4, 2))  # [core,E,HJ,128,IS]
    w3b = np.asarray(w3_weight, dtype=bf16).reshape(E, NCORES, IS, HJ, 128)
    w3c = np.ascontiguousarray(w3b.transpose(1, 0, 3, 4, 2))
    # w2: [E, H, I] -> per core [E, ISC, 128, H]
    w2b = np.asarray(w2_weight, dtype=bf16).reshape(E, H, NCORES, ISC, 128)
    w2c = np.ascontiguousarray(w2b.transpose(2, 0, 3, 4, 1))  # [core,E,ISC,128,H]

    in_maps = [
        {"xt": xt, "w1": w1c[j], "w3": w3c[j], "w2": w2c[j]} for j in range(NCORES)
    ]

    nc = _build_program(tuple(counts))
    trace = os.environ.get("BASS_MOE_TRACE", "") == "1"
    res = None
    if trace:
        import concourse.bass_utils as bu

        orig_upload = bu.upload_artifacts
        bu.upload_artifacts = lambda tmpdir: f"local://{tmpdir}"
        tdir = os.environ.get("BASS_MOE_TRACE_DIR") or None
        try:
            res = run_bass_kernel_spmd(
                nc, in_maps, list(range(NCORES)), trace=True, tmpdir=tdir
            )
        except Exception as exc:
            print(f"trace path failed ({type(exc).__name__}: {exc}); rerunning untraced", flush=True)
            res = None
        finally:
            bu.upload_artifacts = orig_upload
    if res is None:
        res = run_bass_kernel_spmd(nc, in_maps, list(range(NCORES)))
    global last_exec_time_ns, last_results
    last_exec_time_ns = res.exec_time_ns
    last_results = res

    # host combine: sum the 8 partial outputs, then weighted scatter-add
    ysum = np.zeros((H, TT), dtype=np.float32)
    for j in range(NCORES):
        ysum += res.results[j]["yt"].reshape(H, TT).astype(np.float32)

    out = np.zeros((T, H), dtype=np.float32)
    for e in range(E):
        n_e = len(tok_ids[e])
        seg = ysum[:, offs[e] : offs[e] + n_e]
        out[tok_ids[e]] += tok_w[e][:, None] * seg.T
    return out


# revision 11
# speedup vs baseline: 1.1823x; 1.0291x over previous
"""Mixtral MoE layer (8 experts, top-2, H=2048, I=7168, T=8192) on 8 trn2 NeuronCores.

Intermediate-sharded SPMD: every core processes ALL experts' routed tokens but
only a 1/8 slice of the intermediate dim (896 of 7168). Phase A computes its
G-slice = silu(x@w1s.T) * (x@w3s.T); phase B contracts the slice against w2s
into a PARTIAL output accumulated fully in PSUM (7-matmul groups). The host
sums the 8 partial outputs and applies routing weights.

Why: per-core work is identical by construction (no max-expert padding — the
per-expert token counts are baked into the program at build time), phase B
needs no vector-engine accumulation, and weights stream per 512-token block so
everything double-buffers. bf16 matmuls, fp32 PSUM, bf16 partial outputs.
"""

import math

import numpy as np
import ml_dtypes

import concourse.bass as bass
import concourse.mybir as mybir
import concourse.tile as tile
from concourse.bass_utils import run_bass_kernel_spmd

H = 2048          # hidden dim
I = 7168          # intermediate dim
E = 8             # experts
NCORES = 8
IS = I // NCORES  # 896 per-core intermediate slice
ISC = IS // 128   # 7 chunks of 128
HJ = H // 128     # 16 hidden chunks of 128
TOPK = 2
TBMAX = 512       # token block (matmul free dim, one PSUM bank)
# i-groups within the 896 slice for w1/w3 streaming (chunks of 256 + tail 128)
IGS = [(0, 256), (256, 256), (512, 256), (768, 128)]

BF16 = mybir.dt.bfloat16
F32 = mybir.dt.float32

last_exec_time_ns = None  # set when BASS_MOE_TRACE=1
last_results = None


def _install_axon_hooks_shim():
    """This image lacks antenv.axon_hooks (needed by run_bass_kernel_spmd
    trace=True). Provide it, with the NTFF profile hook driven via ctypes
    into the injected axon .so (mirrors trn_agent_boot._ntff_profile_via_ctypes)."""
    import sys

    try:
        import antenv.axon_hooks  # noqa: F401

        return
    except ImportError:
        pass
    import contextlib
    import ctypes
    import types

    hook = None
    so_path = "/opt/axon/libaxon_pjrt.so"
    try:
        lib = ctypes.CDLL(so_path)
        if hasattr(lib, "axon_start_nrt_profile"):
            lib.axon_start_nrt_profile.argtypes = [
                ctypes.POINTER(ctypes.c_int64),
                ctypes.c_size_t,
            ]
            lib.axon_start_nrt_profile.restype = ctypes.c_int64
            lib.axon_stop_nrt_profile.argtypes = [ctypes.c_char_p]
            lib.axon_stop_nrt_profile.restype = ctypes.c_int64

            @contextlib.contextmanager
            def _hook(output_dir, device_ids):
                import jax

                jax.devices()
                if device_ids:
                    ids = (ctypes.c_int64 * len(device_ids))(*device_ids)
                    rc = lib.axon_start_nrt_profile(ids, len(device_ids))
                else:
                    rc = lib.axon_start_nrt_profile(None, 0)
                if rc != 0:
                    raise RuntimeError(f"axon_start_nrt_profile rc={rc}")
                try:
                    yield
                finally:
                    n = lib.axon_stop_nrt_profile(str(output_dir).encode())
                    print(f"ntff profile: {n} file(s) -> {output_dir}", flush=True)

            hook = _hook
    except OSError:
        pass

    mod = types.ModuleType("antenv.axon_hooks")
    mod._hook = hook
    mod.get_axon_ntff_profile_hook = lambda: mod._hook
    mod.set_axon_ntff_profile_hook = lambda h: setattr(mod, "_hook", h)
    sys.modules["antenv.axon_hooks"] = mod


_install_axon_hooks_shim()


def legalize_single_wait(nc):
    """This walrus rejects >1 sem wait per instruction: hoist extras onto
    preceding NoOps on the same engine (per-engine program order preserved)."""
    n_split = 0
    for fn in nc.m.functions:
        for blk in fn.blocks:
            new = []
            for inst in blk.instructions:
                si = inst.sync_info
                if si is not None and si.on_wait and len(si.on_wait) > 1:
                    waits = list(si.on_wait)
                    for i, w in enumerate(waits[:-1]):
                        nop = mybir.InstNoOp(name=f"{inst.name}-w{i}", ins=[], outs=[])
                        nop.engine = inst.engine
                        nop.sync_info = mybir.SyncInfo(on_wait=[w], on_update=[])
                        new.append(nop)
                        n_split += 1
                    inst.sync_info = mybir.SyncInfo(
                        on_wait=[waits[-1]], on_update=list(si.on_update)
                    )
                new.append(inst)
            blk.instructions[:] = new
    return n_split


def _block_sizes(n):
    """Near-equal 16-aligned blocks <= TBMAX covering n (n a multiple of 16)."""
    if n == 0:
        return []
    nb = math.ceil(n / TBMAX)
    base = (n // nb) // 16 * 16
    rem = (n - base * nb) // 16
    return [base + (16 if i < rem else 0) for i in range(nb)]


_programs = {}


def _build_program(counts):
    """One SPMD program: all experts' FFN on a 1/8 inter-slice.

    counts: tuple of 8 padded (mult-of-16) per-expert token counts."""
    key = tuple(counts)
    if key in _programs:
        return _programs[key]

    TT = sum(counts)
    nc = bass.Bass("TRN2", target_bir_lowering=False, debug=False, num_devices=NCORES)
    xt = nc.declare_dram_parameter("xt", [HJ, 128, TT], BF16, isOutput=False)
    w1 = nc.declare_dram_parameter("w1", [E, HJ, 128, IS], BF16, isOutput=False)
    w3 = nc.declare_dram_parameter("w3", [E, HJ, 128, IS], BF16, isOutput=False)
    w2 = nc.declare_dram_parameter("w2", [E, ISC, 128, H], BF16, isOutput=False)
    yt = nc.declare_dram_parameter("yt", [HJ, 128, TT], BF16, isOutput=True)

    # flat per-expert blocks, each its own weight-streaming pass; expert 0
    # leads with a small block so the first matmul waits on ~1.5MB of DMA
    sbs = []
    off = 0
    for e in range(E):
        n = counts[e]
        groups = [256] + _block_sizes(n - 256) if e == 0 and n > 512 else _block_sizes(n)
        for tb in groups:
            sbs.append((e, off, [tb]))
            off += tb
    assert off == TT

    with tile.TileContext(nc) as tc:
        with (
            tc.tile_pool(name="xp", bufs=3) as xp,
            tc.tile_pool(name="w1p", bufs=5) as w1p,
            tc.tile_pool(name="w3p", bufs=5) as w3p,
            tc.tile_pool(name="w2p", bufs=3) as w2p,
            tc.tile_pool(name="gtp", bufs=2 * ISC) as gtp,
            tc.tile_pool(name="sip", bufs=3) as sip,
            tc.tile_pool(name="otp", bufs=6) as otp,
            tc.tile_pool(name="pga", bufs=2, space="PSUM") as pga,
            tc.tile_pool(name="pob", bufs=4, space="PSUM") as pob,
        ):
            first_sb = sbs[0]
            for e, c0, groups in sbs:
                is_first = (e, c0, groups) == first_sb
                sbw = sum(groups)
                goffs = [c0 + sum(groups[:i]) for i in range(len(groups))]
                xsb = xp.tile([128, HJ, sbw], BF16, tag="xsb")
                nc.sync.dma_start(
                    out=xsb[:, :, :],
                    in_=xt[:, :, c0 : c0 + sbw].rearrange("j p c -> p j c"),
                )

                # ---- phase A: G slice [896, sbw]; one weight pass feeds
                # every column group ----
                gts = [[] for _ in groups]
                for i0, gw in IGS:
                    w1sb = w1p.tile([128, HJ, 256], BF16, tag="w1sb")
                    (nc.scalar if is_first else nc.sync).dma_start(
                        out=w1sb[:, :, :gw],
                        in_=w1[e, :, :, i0 : i0 + gw].rearrange("j p i -> p j i"),
                    )
                    w3sb = w3p.tile([128, HJ, 256], BF16, tag="w3sb")
                    (nc.gpsimd if is_first else nc.scalar).dma_start(
                        out=w3sb[:, :, :gw],
                        in_=w3[e, :, :, i0 : i0 + gw].rearrange("j p i -> p j i"),
                    )
                    for m in range(gw // 128):
                        ms = slice(m * 128, (m + 1) * 128)
                        for g, tb in enumerate(groups):
                            cs = slice(goffs[g] - c0, goffs[g] - c0 + tb)
                            pg1 = pga.tile([128, tb], F32, tag="pg1")
                            pg3 = pga.tile([128, tb], F32, tag="pg3")
                            for k in range(HJ):
                                nc.tensor.matmul(
                                    pg1[:, :],
                                    lhsT=w1sb[:, k, ms],
                                    rhs=xsb[:, k, cs],
                                    start=(k == 0),
                                    stop=(k == HJ - 1),
                                )
                            for k in range(HJ):
                                nc.tensor.matmul(
                                    pg3[:, :],
                                    lhsT=w3sb[:, k, ms],
                                    rhs=xsb[:, k, cs],
                                    start=(k == 0),
                                    stop=(k == HJ - 1),
                                )
                            ssb = sip.tile([128, tb], F32, tag="ssb")
                            nc.scalar.activation(
                                ssb[:, :], pg1[:, :], mybir.ActivationFunctionType.Silu
                            )
                            gt = gtp.tile([128, tb], BF16, tag="gt")
                            nc.vector.tensor_mul(gt[:, :], pg3[:, :], ssb[:, :])
                            gts[g].append(gt)

                # ---- phase B: partial Y[2048, sbw], contraction over the 896
                # slice entirely in PSUM (7-matmul groups) ----
                for hg in range(4):
                    w2sb = w2p.tile([128, ISC, 512], BF16, tag="w2sb")
                    nc.gpsimd.dma_start(
                        out=w2sb[:, :, :],
                        in_=w2[e, :, :, hg * 512 : (hg + 1) * 512].rearrange(
                            "c p h -> p c h"
                        ),
                    )
                    for g, tb in enumerate(groups):
                        for hh in range(4):
                            po = pob.tile([128, tb], F32, tag="po")
                            hs = slice(hh * 128, (hh + 1) * 128)
                            for u in range(ISC):
                                nc.tensor.matmul(
                                    po[:, :],
                                    lhsT=w2sb[:, u, hs],
                                    rhs=gts[g][u][:, :],
                                    start=(u == 0),
                                    stop=(u == ISC - 1),
                                )
                            ot = otp.tile([128, tb], BF16, tag="ot")
                            nc.vector.tensor_copy(ot[:, :], po[:, :])
                            nc.gpsimd.dma_start(
                                out=yt[hg * 4 + hh, :, goffs[g] : goffs[g] + tb],
                                in_=ot[:, :],
                            )

    legalize_single_wait(nc)
    _programs[key] = nc
    return nc


def _routing(x, gate_weight):
    """Replicate the reference router bitwise-closely: jax on CPU, same ops."""
    import jax
    import jax.numpy as jnp

    cpu = jax.devices("cpu")[0]
    with jax.default_device(cpu):
        router_logits = jnp.asarray(x) @ jnp.asarray(gate_weight).T
        probs = jax.nn.softmax(router_logits.astype(jnp.float32), axis=-1)
        top_w, top_idx = jax.lax.top_k(probs, TOPK)
        top_w = top_w / jnp.sum(top_w, axis=-1, keepdims=True)
        top_w = top_w.astype(x.dtype)
        return np.asarray(top_w), np.asarray(top_idx)


def kernel(hidden_states, gate_weight, w1_weight, w3_weight, w2_weight):
    import os

    bf16 = ml_dtypes.bfloat16
    x = np.asarray(hidden_states, dtype=np.float32)
    T = x.shape[0]
    top_w, top_idx = _routing(x, np.asarray(gate_weight, dtype=np.float32))

    tok_ids = []
    tok_w = []
    counts = []
    for e in range(E):
        rows, cols = np.nonzero(top_idx == e)
        tok_ids.append(rows)
        tok_w.append(top_w[rows, cols].astype(np.float32))
        counts.append(max(16, math.ceil(len(rows) / 16) * 16))
    TT = sum(counts)
    offs = np.cumsum([0] + counts)

    # gathered, expert-concatenated tokens, transposed for contiguous DMA runs
    xcat = np.zeros((TT, H), dtype=bf16)
    for e in range(E):
        xcat[offs[e] : offs[e] + len(tok_ids[e])] = x[tok_ids[e]]
    xt = np.ascontiguousarray(xcat.T).reshape(HJ, 128, TT)

    # per-core I-slices of all experts' weights, pre-transposed
    # w1/w3: [E, I, H] -> per core [E, HJ, 128, IS]
    w1b = np.asarray(w1_weight, dtype=bf16).reshape(E, NCORES, IS, HJ, 128)
    w1c = np.ascontiguousarray(w1b.transpose(1, 0, 3, 4, 2))  # [core,E,HJ,128,IS]
    w3b = np.asarray(w3_weight, dtype=bf16).reshape(E, NCORES, IS, HJ, 128)
    w3c = np.ascontiguousarray(w3b.transpose(1, 0, 3, 4, 2))
    # w2: [E, H, I] -> per core [E, ISC, 128, H]
    w2b = np.asarray(w2_weight, dtype=bf16).reshape(E, H, NCORES, ISC, 128)
    w2c = np.ascontiguousarray(w2b.transpose(2, 0, 3, 4, 1))  # [core,E,ISC,128,H]

    in_maps = [
        {"xt": xt, "w1": w1c[j], "w3": w3c[j], "w2": w2c[j]} for j in range(NCORES)
    ]

    nc = _build_program(tuple(counts))
    trace = os.environ.get("BASS_MOE_TRACE", "") == "1"
    res = None
    if trace:
        import concourse.bass_utils as bu

        orig_upload = bu.upload_artifacts
        bu.upload_artifacts = lambda tmpdir: f"local://{tmpdir}"
        tdir = os.environ.get("BASS_MOE_TRACE_DIR") or None
        try:
            res = run_bass_kernel_spmd(
                nc, in_maps, list(range(NCORES)), trace=True, tmpdir=tdir
            )
        except Exception as exc:
            print(f"trace path failed ({type(exc).__name__}: {exc}); rerunning untraced", flush=True)
            res = None
        finally:
            bu.upload_artifacts = orig_upload
    if res is None:
        res = run_bass_kernel_spmd(nc, in_maps, list(range(NCORES)))
    global last_exec_time_ns, last_results
    last_exec_time_ns = res.exec_time_ns
    last_results = res

    # host combine: sum the 8 partial outputs, then weighted scatter-add
    ysum = np.zeros((H, TT), dtype=np.float32)
    for j in range(NCORES):
        ysum += res.results[j]["yt"].reshape(H, TT).astype(np.float32)

    out = np.zeros((T, H), dtype=np.float32)
    for e in range(E):
        n_e = len(tok_ids[e])
        seg = ysum[:, offs[e] : offs[e] + n_e]
        out[tok_ids[e]] += tok_w[e][:, None] * seg.T
    return out


# revision 12
# speedup vs baseline: 1.1985x; 1.0137x over previous
"""Mixtral MoE layer (8 experts, top-2, H=2048, I=7168, T=8192) on 8 trn2 NeuronCores.

Intermediate-sharded SPMD: every core processes ALL experts' routed tokens but
only a 1/8 slice of the intermediate dim (896 of 7168). Phase A computes its
G-slice = silu(x@w1s.T) * (x@w3s.T); phase B contracts the slice against w2s
into a PARTIAL output accumulated fully in PSUM (7-matmul groups). The host
sums the 8 partial outputs and applies routing weights.

Why: per-core work is identical by construction (no max-expert padding — the
per-expert token counts are baked into the program at build time), phase B
needs no vector-engine accumulation, and weights stream per 512-token block so
everything double-buffers. bf16 matmuls, fp32 PSUM, bf16 partial outputs.
"""

import math

import numpy as np
import ml_dtypes

import concourse.bass as bass
import concourse.mybir as mybir
import concourse.tile as tile
from concourse.bass_utils import run_bass_kernel_spmd

H = 2048          # hidden dim
I = 7168          # intermediate dim
E = 8             # experts
NCORES = 8
IS = I // NCORES  # 896 per-core intermediate slice
ISC = IS // 128   # 7 chunks of 128
HJ = H // 128     # 16 hidden chunks of 128
TOPK = 2
TBMAX = 512       # token block (matmul free dim, one PSUM bank)
# i-groups within the 896 slice for w1/w3 streaming (chunks of 256 + tail 128)
IGS = [(0, 256), (256, 256), (512, 256), (768, 128)]

BF16 = mybir.dt.bfloat16
F32 = mybir.dt.float32

last_exec_time_ns = None  # set when BASS_MOE_TRACE=1
last_results = None


def _install_axon_hooks_shim():
    """This image lacks antenv.axon_hooks (needed by run_bass_kernel_spmd
    trace=True). Provide it, with the NTFF profile hook driven via ctypes
    into the injected axon .so (mirrors trn_agent_boot._ntff_profile_via_ctypes)."""
    import sys

    try:
        import antenv.axon_hooks  # noqa: F401

        return
    except ImportError:
        pass
    import contextlib
    import ctypes
    import types

    hook = None
    so_path = "/opt/axon/libaxon_pjrt.so"
    try:
        lib = ctypes.CDLL(so_path)
        if hasattr(lib, "axon_start_nrt_profile"):
            lib.axon_start_nrt_profile.argtypes = [
                ctypes.POINTER(ctypes.c_int64),
                ctypes.c_size_t,
            ]
            lib.axon_start_nrt_profile.restype = ctypes.c_int64
            lib.axon_stop_nrt_profile.argtypes = [ctypes.c_char_p]
            lib.axon_stop_nrt_profile.restype = ctypes.c_int64

            @contextlib.contextmanager
            def _hook(output_dir, device_ids):
                import jax

                jax.devices()
                if device_ids:
                    ids = (ctypes.c_int64 * len(device_ids))(*device_ids)
                    rc = lib.axon_start_nrt_profile(ids, len(device_ids))
                else:
                    rc = lib.axon_start_nrt_profile(None, 0)
                if rc != 0:
                    raise RuntimeError(f"axon_start_nrt_profile rc={rc}")
                try:
                    yield
                finally:
                    n = lib.axon_stop_nrt_profile(str(output_dir).encode())
                    print(f"ntff profile: {n} file(s) -> {output_dir}", flush=True)

            hook = _hook
    except OSError:
        pass

    mod = types.ModuleType("antenv.axon_hooks")
    mod._hook = hook
    mod.get_axon_ntff_profile_hook = lambda: mod._hook
    mod.set_axon_ntff_profile_hook = lambda h: setattr(mod, "_hook", h)
    sys.modules["antenv.axon_hooks"] = mod


_install_axon_hooks_shim()


def legalize_single_wait(nc):
    """This walrus rejects >1 sem wait per instruction: hoist extras onto
    preceding NoOps on the same engine (per-engine program order preserved)."""
    n_split = 0
    for fn in nc.m.functions:
        for blk in fn.blocks:
            new = []
            for inst in blk.instructions:
                si = inst.sync_info
                if si is not None and si.on_wait and len(si.on_wait) > 1:
                    waits = list(si.on_wait)
                    for i, w in enumerate(waits[:-1]):
                        nop = mybir.InstNoOp(name=f"{inst.name}-w{i}", ins=[], outs=[])
                        nop.engine = inst.engine
                        nop.sync_info = mybir.SyncInfo(on_wait=[w], on_update=[])
                        new.append(nop)
                        n_split += 1
                    inst.sync_info = mybir.SyncInfo(
                        on_wait=[waits[-1]], on_update=list(si.on_update)
                    )
                new.append(inst)
            blk.instructions[:] = new
    return n_split


def _block_sizes(n):
    """Near-equal 16-aligned blocks <= TBMAX covering n (n a multiple of 16)."""
    if n == 0:
        return []
    nb = math.ceil(n / TBMAX)
    base = (n // nb) // 16 * 16
    rem = (n - base * nb) // 16
    return [base + (16 if i < rem else 0) for i in range(nb)]


_programs = {}


def _build_program(counts):
    """One SPMD program: all experts' FFN on a 1/8 inter-slice.

    counts: tuple of 8 padded (mult-of-16) per-expert token counts."""
    key = tuple(counts)
    if key in _programs:
        return _programs[key]

    TT = sum(counts)
    nc = bass.Bass("TRN2", target_bir_lowering=False, debug=False, num_devices=NCORES)
    xt = nc.declare_dram_parameter("xt", [HJ, 128, TT], BF16, isOutput=False)
    w1 = nc.declare_dram_parameter("w1", [E, HJ, 128, IS], BF16, isOutput=False)
    w3 = nc.declare_dram_parameter("w3", [E, HJ, 128, IS], BF16, isOutput=False)
    w2 = nc.declare_dram_parameter("w2", [E, ISC, 128, H], BF16, isOutput=False)
    yt = nc.declare_dram_parameter("yt", [HJ, 128, TT], BF16, isOutput=True)

    # flat per-expert blocks, each its own weight-streaming pass
    sbs = []
    off = 0
    for e in range(E):
        n = counts[e]
        groups = _block_sizes(n)
        for tb in groups:
            sbs.append((e, off, [tb]))
            off += tb
    assert off == TT

    with tile.TileContext(nc) as tc:
        with (
            tc.tile_pool(name="xp", bufs=2) as xp,
            tc.tile_pool(name="w1p", bufs=3) as w1p,
            tc.tile_pool(name="w3p", bufs=3) as w3p,
            tc.tile_pool(name="w2p", bufs=3) as w2p,
            tc.tile_pool(name="gtp", bufs=2 * ISC) as gtp,
            tc.tile_pool(name="sip", bufs=3) as sip,
            tc.tile_pool(name="otp", bufs=6) as otp,
            tc.tile_pool(name="pga", bufs=2, space="PSUM") as pga,
            tc.tile_pool(name="pob", bufs=4, space="PSUM") as pob,
        ):
            for e, c0, groups in sbs:
                sbw = sum(groups)
                goffs = [c0 + sum(groups[:i]) for i in range(len(groups))]
                xsb = xp.tile([128, HJ, sbw], BF16, tag="xsb")
                nc.sync.dma_start(
                    out=xsb[:, :, :],
                    in_=xt[:, :, c0 : c0 + sbw].rearrange("j p c -> p j c"),
                )

                # ---- phase A: G slice [896, sbw]; one weight pass feeds
                # every column group ----
                gts = [[] for _ in groups]
                for i0, gw in IGS:
                    w1sb = w1p.tile([128, HJ, 256], BF16, tag="w1sb")
                    nc.sync.dma_start(
                        out=w1sb[:, :, :gw],
                        in_=w1[e, :, :, i0 : i0 + gw].rearrange("j p i -> p j i"),
                    )
                    w3sb = w3p.tile([128, HJ, 256], BF16, tag="w3sb")
                    nc.scalar.dma_start(
                        out=w3sb[:, :, :gw],
                        in_=w3[e, :, :, i0 : i0 + gw].rearrange("j p i -> p j i"),
                    )
                    for m in range(gw // 128):
                        ms = slice(m * 128, (m + 1) * 128)
                        for g, tb in enumerate(groups):
                            cs = slice(goffs[g] - c0, goffs[g] - c0 + tb)
                            pg1 = pga.tile([128, tb], F32, tag="pg1")
                            pg3 = pga.tile([128, tb], F32, tag="pg3")
                            for k in range(HJ):
                                nc.tensor.matmul(
                                    pg1[:, :],
                                    lhsT=w1sb[:, k, ms],
                                    rhs=xsb[:, k, cs],
                                    start=(k == 0),
                                    stop=(k == HJ - 1),
                                )
                            for k in range(HJ):
                                nc.tensor.matmul(
                                    pg3[:, :],
                                    lhsT=w3sb[:, k, ms],
                                    rhs=xsb[:, k, cs],
                                    start=(k == 0),
                                    stop=(k == HJ - 1),
                                )
                            ssb = sip.tile([128, tb], F32, tag="ssb")
                            nc.scalar.activation(
                                ssb[:, :], pg1[:, :], mybir.ActivationFunctionType.Silu
                            )
                            gt = gtp.tile([128, tb], BF16, tag="gt")
                            nc.vector.tensor_mul(gt[:, :], pg3[:, :], ssb[:, :])
                            gts[g].append(gt)

                # ---- phase B: partial Y[2048, sbw], contraction over the 896
                # slice entirely in PSUM (7-matmul groups) ----
                for hg in range(4):
                    w2sb = w2p.tile([128, ISC, 512], BF16, tag="w2sb")
                    nc.gpsimd.dma_start(
                        out=w2sb[:, :, :],
                        in_=w2[e, :, :, hg * 512 : (hg + 1) * 512].rearrange(
                            "c p h -> p c h"
                        ),
                    )
                    for g, tb in enumerate(groups):
                        for hh in range(4):
                            po = pob.tile([128, tb], F32, tag="po")
                            hs = slice(hh * 128, (hh + 1) * 128)
                            for u in range(ISC):
                                nc.tensor.matmul(
                                    po[:, :],
                                    lhsT=w2sb[:, u, hs],
                                    rhs=gts[g][u][:, :],
                                    start=(u == 0),
                                    stop=(u == ISC - 1),
                                )
                            ot = otp.tile([128, tb], BF16, tag="ot")
                            nc.vector.tensor_copy(ot[:, :], po[:, :])
                            nc.gpsimd.dma_start(
                                out=yt[hg * 4 + hh, :, goffs[g] : goffs[g] + tb],
                                in_=ot[:, :],
                            )

    legalize_single_wait(nc)
    _programs[key] = nc
    return nc


def _routing(x, gate_weight):
    """Replicate the reference router bitwise-closely: jax on CPU, same ops."""
    import jax
    import jax.numpy as jnp

    cpu = jax.devices("cpu")[0]
    with jax.default_device(cpu):
        router_logits = jnp.asarray(x) @ jnp.asarray(gate_weight).T
        probs = jax.nn.softmax(router_logits.astype(jnp.float32), axis=-1)
        top_w, top_idx = jax.lax.top_k(probs, TOPK)
        top_w = top_w / jnp.sum(top_w, axis=-1, keepdims=True)
        top_w = top_w.astype(x.dtype)
        return np.asarray(top_w), np.asarray(top_idx)


def kernel(hidden_states, gate_weight, w1_weight, w3_weight, w2_weight):
    import os

    bf16 = ml_dtypes.bfloat16
    x = np.asarray(hidden_states, dtype=np.float32)
    T = x.shape[0]
    top_w, top_idx = _routing(x, np.asarray(gate_weight, dtype=np.float32))

    tok_ids = []
    tok_w = []
    counts = []
    for e in range(E):
        rows, cols = np.nonzero(top_idx == e)
        tok_ids.append(rows)
        tok_w.append(top_w[rows, cols].astype(np.float32))
        counts.append(max(16, math.ceil(len(rows) / 16) * 16))
    TT = sum(counts)
    offs = np.cumsum([0] + counts)

    # gathered, expert-concatenated tokens, transposed for contiguous DMA runs
    xcat = np.zeros((TT, H), dtype=bf16)
    for e in range(E):
        xcat[offs[e] : offs[e] + len(tok_ids[e])] = x[tok_ids[e]]
    xt = np.ascontiguousarray(xcat.T).reshape(HJ, 128, TT)

    # per-core I-slices of all experts' weights, pre-transposed
    # w1/w3: [E, I, H] -> per core [E, HJ, 128, IS]
    w1b = np.asarray(w1_weight, dtype=bf16).reshape(E, NCORES, IS, HJ, 128)
    w1c = np.ascontiguousarray(w1b.transpose(1, 0, 3, 4, 2))  # [core,E,HJ,128,IS]
    w3b = np.asarray(w3_weight, dtype=bf16).reshape(E, NCORES, IS, HJ, 128)
    w3c = np.ascontiguousarray(w3b.transpose(1, 0, 3, 4, 2))
    # w2: [E, H, I] -> per core [E, ISC, 128, H]
    w2b = np.asarray(w2_weight, dtype=bf16).reshape(E, H, NCORES, ISC, 128)
    w2c = np.ascontiguousarray(w2b.transpose(2, 0, 3, 4, 1))  # [core,E,ISC,128,H]

    in_maps = [
        {"xt": xt, "w1": w1c[j], "w3": w3c[j], "w2": w2c[j]} for j in range(NCORES)
    ]

    nc = _build_program(tuple(counts))
    trace = os.environ.get("BASS_MOE_TRACE", "") == "1"
    res = None
    if trace:
        import concourse.bass_utils as bu

        orig_upload = bu.upload_artifacts
        bu.upload_artifacts = lambda tmpdir: f"local://{tmpdir}"
        tdir = os.environ.get("BASS_MOE_TRACE_DIR") or None
        try:
            res = run_bass_kernel_spmd(
                nc, in_maps, list(range(NCORES)), trace=True, tmpdir=tdir
            )
        except Exception as exc:
            print(f"trace path failed ({type(exc).__name__}: {exc}); rerunning untraced", flush=True)
            res = None
        finally:
            bu.upload_artifacts = orig_upload
    if res is None:
        res = run_bass_kernel_spmd(nc, in_maps, list(range(NCORES)))
    global last_exec_time_ns, last_results
    last_exec_time_ns = res.exec_time_ns
    last_results = res

    # host combine: sum the 8 partial outputs, then weighted scatter-add
    ysum = np.zeros((H, TT), dtype=np.float32)
    for j in range(NCORES):
        ysum += res.results[j]["yt"].reshape(H, TT).astype(np.float32)

    out = np.zeros((T, H), dtype=np.float32)
    for e in range(E):
        n_e = len(tok_ids[e])
        seg = ysum[:, offs[e] : offs[e] + n_e]
        out[tok_ids[e]] += tok_w[e][:, None] * seg.T
    return out
